# revision 11
# baseline (speedup 1.0000x reference)
"""Trainium2 Bass kernel for CAML-style label-wise-attention CNN.

Model (per batch element b):
    emb = W_embed[x[b]]                      # [L, E]
    h   = tanh(conv1d(emb, conv_w) + conv_b) # [L, F]
    S   = h @ U_w.T                          # [L, Y]
    alpha = softmax_L(S)
    m   = alpha.T @ h                        # [Y, F]
    y   = sum_f(final_w * m) + final_b       # [Y]
    loss = mean BCEWithLogits(y, target)

Sharding: data-parallel over batch (B=8 -> one batch element per NeuronCore).
The [Y, L] score matrix is never materialized in HBM: scores are computed in
PSUM in [128 (L), 1024 (Y)] tiles, exponentiated on the scalar engine into
SBUF, and immediately pooled back through the tensor engine.  A ones-column
appended to h makes the same pooling matmul produce the softmax denominator.

Self-contained: hardcodes all shapes; only needs numpy + concourse (bass).
"""

import numpy as np

import concourse.bacc as bacc
import concourse.tile as tile
from concourse import bass, mybir
from concourse.bass import IndirectOffsetOnAxis
from concourse.bass_utils import run_bass_kernel_spmd
from concourse.masks import make_identity

V, E, L, B, F, Y, K = 50000, 100, 2500, 8, 50, 8921, 9
PAD = K // 2
LP = 2560           # tokens padded to 20*128 (pad index 0 -> zero embedding row)
NLT = LP // 128     # 20 L-tiles
YP = 9216           # labels padded to 9*1024
NYC = YP // 1024    # 9 Y-chunks
CONV_CHUNKS = [(0, 512), (512, 512), (1024, 512), (1536, 512), (2048, 452)]
DCOL = 64           # ones-column index in h1 (denominator row; must be 32-aligned)

F32 = mybir.dt.float32
F32R = mybir.dt.float32r

_NC_CACHE = {}


def _lsz(l):
    """valid token count in L-tile l (last tile is partial: 2500 = 19*128+68)"""
    return min(128, L - l * 128)


def _build_nc(use_r=True):
    """Build the Bass module. use_r: float32r (full-rate fp32) matmul operands."""
    MMDT = F32R if use_r else F32
    nc = bacc.Bacc("TRN2", target_bir_lowering=False)

    d_emb = nc.dram_tensor("w_embed", [V, E], F32, kind="ExternalInput")
    d_idx = nc.dram_tensor("idx", [128, NLT], mybir.dt.int32, kind="ExternalInput")
    d_ut = nc.dram_tensor("u_wt", [F, YP], MMDT, kind="ExternalInput")
    d_ft = nc.dram_tensor("f_wt", [F, YP], F32, kind="ExternalInput")
    d_cw = nc.dram_tensor("conv_w", [E, K, F], MMDT, kind="ExternalInput")
    d_cb = nc.dram_tensor("conv_b", [F, 1], F32, kind="ExternalInput")
    d_y = nc.dram_tensor("y", [1, YP], F32, kind="ExternalOutput")

    with tile.TileContext(nc) as tc:
        with (
            tc.tile_pool(name="const", bufs=1) as const,
            tc.tile_pool(name="gat", bufs=4) as gat,
            tc.tile_pool(name="pt", bufs=2) as ptp,
            tc.tile_pool(name="fin", bufs=1) as fin,
            tc.tile_pool(name="psS", bufs=3, space="PSUM") as psS,
            tc.tile_pool(name="psM", bufs=1, space="PSUM") as psM,
        ):
            ident = const.tile([128, 128], F32, tag="ident")
            make_identity(nc, ident[:])
            idx_sb = const.tile([128, NLT], mybir.dt.int32, tag="idx")
            nc.sync.dma_start(out=idx_sb[:], in_=d_idx[:])
            ut_sb = const.tile([F, YP], MMDT, tag="ut")
            nc.sync.dma_start(out=ut_sb[:], in_=d_ut[:])
            ft_sb = const.tile([F, YP], F32, tag="ft")
            nc.sync.dma_start(out=ft_sb[:], in_=d_ft[:])
            cw_sb = const.tile([E, K, F], MMDT, tag="cw")
            nc.sync.dma_start(out=cw_sb[:], in_=d_cw[:])
            cb_sb = const.tile([F, 1], F32, tag="cb")
            nc.sync.dma_start(out=cb_sb[:], in_=d_cb[:])
            onesF = const.tile([128, 1], F32, tag="onesF")
            nc.vector.memset(onesF[:], 1.0)
            zerosF = const.tile([128, DCOL - F], F32, tag="zerosF")
            nc.vector.memset(zerosF[:], 0.0)
            ones_sb = const.tile([F, 1], MMDT, tag="ones")
            nc.vector.tensor_copy(out=ones_sb[:], in_=onesF[:F, :])

            # ---- stage A: embedding gather + transpose -> embT [E, 4+LP] ----
            embT = const.tile([E, LP + 4], MMDT, tag="embT")
            nc.vector.tensor_copy(out=embT[:, 0:4], in_=zerosF[:E, :4])
            for i in range(NLT):
                ge = gat.tile([128, E], F32, tag="ge")
                nc.gpsimd.indirect_dma_start(
                    out=ge[:],
                    out_offset=None,
                    in_=d_emb[:],
                    in_offset=IndirectOffsetOnAxis(ap=idx_sb[:, i : i + 1], axis=0),
                )
                tp = psS.tile([E, 128], F32, tag="S")
                nc.tensor.transpose(out=tp[:, :], in_=ge[:, :], identity=ident[:])
                nc.vector.tensor_copy(
                    out=embT[:, 4 + 128 * i : 4 + 128 * (i + 1)], in_=tp[:, :]
                )

            # ---- stage B: conv + tanh -> hT [F, L] (f32) + hT_r (matmul dt) ----
            hT = const.tile([F, L], F32, tag="hT")
            for c0, cw in CONV_CHUNKS:
                hc = psS.tile([F, 512], F32, tag="S")
                for k in range(K):
                    nc.tensor.matmul(
                        out=hc[:, :cw],
                        lhsT=cw_sb[:, k, :],
                        rhs=embT[:, c0 + k : c0 + k + cw],
                        start=(k == 0),
                        stop=(k == K - 1),
                    )
                nc.scalar.activation(
                    out=hT[:, c0 : c0 + cw],
                    in_=hc[:, :cw],
                    func=mybir.ActivationFunctionType.Tanh,
                    bias=cb_sb[:, :1],
                    scale=1.0,
                )
            if use_r:
                hT_r = const.tile([F, L], F32R, tag="hTr")
                nc.vector.tensor_copy(out=hT_r[:], in_=hT[:])
            else:
                hT_r = hT

            # ---- stage B2: h1 tiles [128, F+1] (h with ones column) ----
            h1 = []
            for l in range(NLT):
                sz = _lsz(l)
                h1_l = const.tile([128, DCOL + 1], MMDT, tag=f"h1_{l}")
                tp2 = psS.tile([128, F], F32, tag="S")
                nc.tensor.transpose(
                    out=tp2[:sz, :],
                    in_=hT[:, l * 128 : l * 128 + sz],
                    identity=ident[:F, :F],
                )
                nc.vector.tensor_copy(out=h1_l[:sz, :F], in_=tp2[:sz, :])
                nc.vector.tensor_copy(out=h1_l[:sz, F:DCOL], in_=zerosF[:sz, :])
                nc.vector.tensor_copy(out=h1_l[:sz, DCOL : DCOL + 1], in_=onesF[:sz, :])
                h1.append(h1_l)

            # ---- stage C: attention / pooling main loop ----
            for yc in range(NYC):
                y0 = yc * 1024
                mt = psM.tile([DCOL + 1, 1024], F32, tag="M")
                for l in range(NLT):
                    sz = _lsz(l)
                    l0 = l * 128
                    st = psS.tile([128, 1024], F32, tag="S")
                    for hb in range(2):
                        nc.tensor.matmul(
                            out=st[:sz, hb * 512 : (hb + 1) * 512],
                            lhsT=hT_r[:, l0 : l0 + sz],
                            rhs=ut_sb[:, y0 + hb * 512 : y0 + (hb + 1) * 512],
                            start=True,
                            stop=True,
                        )
                    pt = ptp.tile([128, 1024], MMDT, tag="PT")
                    nc.scalar.activation(
                        out=pt[:sz, :],
                        in_=st[:sz, :],
                        func=mybir.ActivationFunctionType.Exp,
                    )
                    for hb in range(2):
                        nc.tensor.matmul(
                            out=mt[:, hb * 512 : (hb + 1) * 512],
                            lhsT=h1[l][:sz, :],
                            rhs=pt[:sz, hb * 512 : (hb + 1) * 512],
                            start=(l == 0),
                            stop=(l == NLT - 1),
                        )
                # finish: y = (sum_f FT*mt) / denom + fb
                prod = fin.tile([F, 1024], MMDT, tag="prod")
                nc.vector.tensor_mul(prod[:, :], mt[:F, :], ft_sb[:, y0 : y0 + 1024])
                recip = fin.tile([1, 1024], F32, tag="recip")
                nc.vector.reciprocal(out=recip[:, :], in_=mt[DCOL : DCOL + 1, :])
                ys = psS.tile([1, 1024], F32, tag="S")
                for hb in range(2):
                    nc.tensor.matmul(
                        out=ys[:, hb * 512 : (hb + 1) * 512],
                        lhsT=ones_sb[:, :],
                        rhs=prod[:, hb * 512 : (hb + 1) * 512],
                        start=True,
                        stop=True,
                    )
                tmp = fin.tile([1, 1024], F32, tag="tmp")
                nc.vector.tensor_mul(tmp[:, :], ys[:1, :], recip[:, :])
                nc.sync.dma_start(out=d_y[:, y0 : y0 + 1024], in_=tmp[:, :])

    nc.finalize()
    return nc


def _get_nc(use_r=True):
    if use_r not in _NC_CACHE:
        _NC_CACHE[use_r] = _build_nc(use_r)
    return _NC_CACHE[use_r]


def _prep_inputs(inputs):
    x = np.asarray(inputs["x"]).astype(np.int32)
    w_embed = np.ascontiguousarray(np.asarray(inputs["W_embed"], dtype=np.float32))
    conv_w = np.asarray(inputs["conv_w"], dtype=np.float32)
    conv_b = np.asarray(inputs["conv_b"], dtype=np.float32)
    u_w = np.asarray(inputs["U_w"], dtype=np.float32)
    final_w = np.asarray(inputs["final_w"], dtype=np.float32)
    final_b = np.asarray(inputs["final_b"], dtype=np.float32)

    x_pad = np.zeros((B, LP), np.int32)
    x_pad[:, :L] = x
    ut = np.zeros((F, YP), np.float32)
    ut[:, :Y] = u_w.T
    ft = np.zeros((F, YP), np.float32)
    ft[:, :Y] = final_w.T
    cw = np.ascontiguousarray(conv_w.transpose(1, 2, 0))  # [E, K, F]
    cb = np.ascontiguousarray(conv_b.reshape(F, 1))

    in_maps = []
    for b in range(B):
        idx = np.ascontiguousarray(x_pad[b].reshape(NLT, 128).T)  # [128, NLT]
        in_maps.append(
            {
                "w_embed": w_embed,
                "idx": idx,
                "u_wt": ut,
                "f_wt": ft,
                "conv_w": cw,
                "conv_b": cb,
            }
        )
    return in_maps, final_b


def _loss_from_y(y, target):
    t = np.asarray(target, dtype=np.float32)
    yf = y.astype(np.float32)
    per = np.maximum(yf, 0.0) - yf * t + np.log1p(np.exp(-np.abs(yf)))
    return np.float32(np.mean(per))


def _run(inputs, use_r=True, **spmd_kwargs):
    nc = _get_nc(use_r)
    in_maps, final_b = _prep_inputs(inputs)
    res = run_bass_kernel_spmd(nc, in_maps, core_ids=list(range(B)), **spmd_kwargs)
    y = np.concatenate([res.results[b]["y"][:, :Y] for b in range(B)], axis=0)
    y = (y + final_b[None, :]).astype(np.float32)
    loss = _loss_from_y(y, inputs["target"])
    return (y, loss), res


def kernel(**inputs):
    (y, loss), _ = _run(inputs)
    return y, loss
